# revision 2
# baseline (speedup 1.0000x reference)
"""DGCN layer (message passing GNN) on 8 Trainium2 NeuronCores via Bass/Tile.

v2 of the dst-sharded gather kernel. Changes vs v1:
  - h table staged in bf16 with the src-side norm (outdeg^-0.5) folded in on
    the host, so the per-edge coefficient reduces to alpha^dist (exact powers
    of two, bf16-exact). Gather rows are 256B instead of 512B.
  - 4 SWDGE queues (ucode serves queue k with Q7 core pair 2k/2k+1, so 4
    queues double descriptor-generation throughput vs 2).
  - The per-tile selection matrices are built 8 tiles at a time with two
    bf16 DVE ops over [128, 8, 128] broadcast APs (is_equal then coef mult),
    amortizing DVE dispatch overhead.
  - Matmuls run in bf16 (G tile as lhsT -> FWL weight loads), PSUM fp32.
  - Phase 2 (agg @ W, scale, bias, store) is interleaved per window.
"""

import math

import numpy as np

P = 128
ALPHA = 0.5
N_CORES = 8
SPLIT = 32768  # int16 index limit for dma_gather
GCH = 8  # tiles per dma_gather (hw limit: <=1024 idxs/inst)
SELG = 8  # tiles per multi-tile sel build


def _wrap_idx16(flat):
    """dma_gather index layout: entry k -> partition k%16, column k//16,
    replicated across the 8 gpsimd core groups (partitions 16-127)."""
    n = flat.shape[-1]
    assert n % 16 == 0
    cols = n // 16
    w = np.asarray(flat, np.int16).reshape(cols, 16).T  # [16, cols]
    return np.tile(w, (8, 1))  # [128, cols]


def _prep_host(h, src, dst, distance, n_cores):
    """Shard edges by dst range; build per-core padded tile arrays."""
    N, D = h.shape
    E = src.shape[0]
    npc = N // n_cores
    n_windows = (npc + P - 1) // P

    src = np.asarray(src).astype(np.int64)
    dst = np.asarray(dst).astype(np.int64)
    distance = np.asarray(distance)

    out_deg = np.bincount(src, minlength=N).astype(np.float64)
    in_deg = np.bincount(dst, minlength=N).astype(np.float64)
    coef_all = np.float64(ALPHA) ** distance.astype(np.float64)
    s_all = in_deg**-1.5  # applied after the W matmul

    # Balanced node -> (core, window, slot) assignment (as v1): deal nodes
    # (sorted by in-degree) into the n_cores*n_windows bins in rounds.
    n_bins = n_cores * n_windows
    lo_deg = np.bincount(dst[src < SPLIT], minlength=N).astype(np.int64)
    hi_deg = np.bincount(dst[src >= SPLIT], minlength=N).astype(np.int64)
    order_nodes = np.argsort(-(lo_deg + hi_deg), kind="stable")
    node_bin = np.empty(N, np.int64)
    node_slot = np.empty(N, np.int64)
    hi_sum = np.zeros(n_bins, np.int64)
    fill = np.zeros(n_bins, np.int64)
    pos = 0
    while pos < N:
        take = min(n_bins, N - pos)
        nodes_r = order_nodes[pos : pos + take]
        nodes_r = nodes_r[np.argsort(-hi_deg[nodes_r], kind="stable")]
        bins_r = np.argsort(hi_sum, kind="stable")[:take]
        node_bin[nodes_r] = bins_r
        node_slot[nodes_r] = fill[bins_r]
        fill[bins_r] += 1
        hi_sum[bins_r] += hi_deg[nodes_r]
        pos += take
    node_core = node_bin // n_windows
    node_window = node_bin % n_windows

    core_of = node_core[dst]
    w_of = node_window[dst]
    r_of = node_slot[dst].astype(np.float32)
    is_hi = (src >= SPLIT).astype(np.int64)

    # sort edges by (core, window, lo/hi) — stable
    gw = (core_of * n_windows + w_of) * 2 + is_hi
    n_gw = n_cores * n_windows * 2
    counts = np.bincount(gw, minlength=n_gw)
    cl = counts.reshape(n_cores, n_windows, 2)
    T_lo = max(1, int(math.ceil(cl[:, :, 0].max() / P)))
    T_hi = max(1, int(math.ceil(cl[:, :, 1].max() / P)))
    T = T_lo + T_hi
    n_cols = n_windows * T

    order = np.argsort(gw, kind="stable")
    sgw = gw[order]
    win_start = np.concatenate([[0], np.cumsum(counts)[:-1]])
    q = np.arange(E, dtype=np.int64) - win_start[sgw]  # pos within group

    core_arr = sgw // (2 * n_windows)
    w_arr = (sgw // 2) % n_windows
    hi_arr = sgw % 2
    j_arr = q // P + hi_arr * T_lo  # hi tiles come after the lo tiles
    p_arr = q % P
    col_arr = w_arr * T + j_arr

    rofs = np.zeros((n_cores, P, n_cols), np.float32)
    coef = np.zeros((n_cores, P, n_cols), np.float32)
    rofs[core_arr, p_arr, col_arr] = r_of[order]
    coef[core_arr, p_arr, col_arr] = coef_all[order].astype(np.float32)

    # int16 gather indices, padded with 0 (coef 0 nullifies), table-relative
    srcrel = np.zeros((n_cores, P, n_cols), np.int64)
    srcrel[core_arr, p_arr, col_arr] = src[order] - (src[order] >= SPLIT) * SPLIT

    # wrapped idx16: per core, per window: lo block then hi block.
    # Blocks start at 64B-aligned column offsets (32 int16 cols).
    CL, CH = T_lo * 8, T_hi * 8  # int16 cols per window per table
    CLa = (CL + 31) // 32 * 32
    CHa = (CH + 31) // 32 * 32
    idx16 = np.zeros((n_cores, P, n_windows * (CLa + CHa)), np.int16)
    for c in range(n_cores):
        flat = srcrel[c].T  # [n_cols, P]: (tile, lane)
        for w in range(n_windows):
            lo = flat[w * T : w * T + T_lo].reshape(-1)
            hi = flat[w * T + T_lo : (w + 1) * T].reshape(-1)
            base = w * (CLa + CHa)
            idx16[c, :, base : base + CL] = _wrap_idx16(lo)
            idx16[c, :, base + CLa : base + CLa + CH] = _wrap_idx16(hi)

    snode = np.ones((n_cores, P, n_windows), np.float32)
    snode[node_core, node_slot, node_window] = s_all.astype(np.float32)

    # host-side inverse permutation: node v lives at core_out row
    # node_window*128 + node_slot of core node_core
    out_core = node_core
    out_row = node_window * P + node_slot

    # src-side norm folded into the bf16 table
    table = (np.asarray(h, np.float64) * (out_deg**-0.5)[:, None]).astype(
        np.float32
    )

    return (
        table, idx16, rofs, coef, snode, out_core, out_row,
        n_windows, T_lo, T_hi, n_cols,
    )


def _build_nc(N, D, n_windows, T_lo, T_hi, n_cols):
    import concourse.bacc as bacc
    import concourse.tile as tile
    from concourse import mybir

    f32 = mybir.dt.float32
    bf16 = mybir.dt.bfloat16
    i16 = mybir.dt.int16
    T = T_lo + T_hi
    CL, CH = T_lo * 8, T_hi * 8
    CLa = (CL + 31) // 32 * 32
    CHa = (CH + 31) // 32 * 32

    # fconst16 free-dim layout (bf16): rofs | coef | iota | wmat
    f16tot = 2 * n_cols + P + D
    # fconst32 free-dim layout (fp32): biasf | snode
    f32tot = D + n_windows

    nc = bacc.Bacc(
        None, target_bir_lowering=False, debug=False, num_swdge_queues=4
    )
    h_d = nc.declare_dram_parameter("h", [N, D], bf16, isOutput=False)
    idx_d = nc.declare_dram_parameter(
        "idx16", [P, n_windows * (CLa + CHa)], i16, isOutput=False
    )
    fc16_d = nc.declare_dram_parameter("fconst16", [P, f16tot], bf16, isOutput=False)
    fc32_d = nc.declare_dram_parameter("fconst32", [P, f32tot], f32, isOutput=False)
    out_d = nc.declare_dram_parameter("out", [n_windows * P, D], f32, isOutput=True)

    mult = mybir.AluOpType.mult

    with tile.TileContext(nc) as tc:
        with (
            tc.tile_pool(name="singles", bufs=1) as singles,
            tc.tile_pool(name="glo", bufs=8) as glopool,
            tc.tile_pool(name="ghi", bufs=4) as ghipool,
            tc.tile_pool(name="sel", bufs=8) as selpool,
            tc.tile_pool(name="agg", bufs=4) as aggpool,
            tc.tile_pool(name="psum", bufs=4, space="PSUM") as psumpool,
            tc.tile_pool(name="psum2", bufs=2, space="PSUM") as psum2pool,
            tc.tile_pool(name="outp", bufs=3) as outpool,
        ):
            idx_sb = singles.tile([P, n_windows * (CLa + CHa)], i16)
            tot = n_windows * (CLa + CHa)
            hd = min(2, n_windows) * (CLa + CHa)
            nc.sync.dma_start(out=idx_sb[:, :hd], in_=idx_d[:, :hd])
            if hd < tot:
                nc.sync.dma_start(out=idx_sb[:, hd:], in_=idx_d[:, hd:])
            fc16_sb = singles.tile([P, f16tot], bf16)
            nc.sync.dma_start(out=fc16_sb[:], in_=fc16_d[:])
            fc32_sb = singles.tile([P, f32tot], f32)
            nc.sync.dma_start(out=fc32_sb[:], in_=fc32_d[:])

            r_sb = fc16_sb[:, 0:n_cols]
            c_sb = fc16_sb[:, n_cols : 2 * n_cols]
            o0 = 2 * n_cols
            io_sb = fc16_sb[:, o0 : o0 + P]
            w_sb = fc16_sb[:, o0 + P : o0 + P + D]
            b_sb = fc32_sb[:, 0:D]
            s_sb = fc32_sb[:, D : D + n_windows]

            h_lo = h_d[0 : min(SPLIT, N), :]
            hi_base = SPLIT if N > SPLIT else 0
            h_hi = h_d[hi_base:N, :]

            n_selg = (T + SELG - 1) // SELG

            qctr = 0
            for w in range(n_windows):
                base = w * (CLa + CHa)
                lo_chunks = []
                for k in range((T_lo + GCH - 1) // GCH):
                    nt = min(GCH, T_lo - k * GCH)
                    g = glopool.tile([P, GCH, P], bf16, tag="glo")
                    cb = base + k * GCH * 8
                    nc.gpsimd.dma_gather(
                        g[:, :nt, :],
                        h_lo,
                        idx_sb[:, cb : cb + nt * 8],
                        nt * P,
                        nt * P,
                        P,
                        single_packet=False,
                        queue_num=qctr % 4,
                    )
                    qctr += 1
                    lo_chunks.append(g)
                hi_chunks = []
                for k in range((T_hi + GCH - 1) // GCH):
                    nt = min(GCH, T_hi - k * GCH)
                    g = ghipool.tile([P, GCH, P], bf16, tag="ghi")
                    cb = base + CLa + k * GCH * 8
                    nc.gpsimd.dma_gather(
                        g[:, :nt, :],
                        h_hi,
                        idx_sb[:, cb : cb + nt * 8],
                        nt * P,
                        nt * P,
                        P,
                        single_packet=False,
                        queue_num=qctr % 4,
                    )
                    qctr += 1
                    hi_chunks.append(g)

                # sel for SELG tiles at a time: two bf16 DVE ops per group
                sel_groups = []
                for sg in range(n_selg):
                    ng = min(SELG, T - sg * SELG)
                    t0 = w * T + sg * SELG
                    sel = selpool.tile([P, SELG, P], bf16, tag="sel")
                    rb = r_sb[:, t0 : t0 + ng].unsqueeze(2).broadcast_to(
                        [P, ng, P]
                    )
                    iob = io_sb.unsqueeze(1).broadcast_to([P, ng, P])
                    nc.vector.tensor_tensor(
                        out=sel[:, :ng, :],
                        in0=rb,
                        in1=iob,
                        op=mybir.AluOpType.is_equal,
                    )
                    cb16 = c_sb[:, t0 : t0 + ng].unsqueeze(2).broadcast_to(
                        [P, ng, P]
                    )
                    nc.vector.tensor_tensor(
                        out=sel[:, :ng, :],
                        in0=sel[:, :ng, :],
                        in1=cb16,
                        op=mult,
                    )
                    sel_groups.append(sel)

                ps = psumpool.tile([P, P], f32)
                for j in range(T):
                    if j < T_lo:
                        lhsT = lo_chunks[j // GCH][:, j % GCH, :]
                    else:
                        jh = j - T_lo
                        lhsT = hi_chunks[jh // GCH][:, jh % GCH, :]
                    rhs = sel_groups[j // SELG][:, j % SELG, :]
                    nc.tensor.matmul(
                        out=ps[:],
                        lhsT=lhsT,
                        rhs=rhs,
                        start=(j == 0),
                        stop=(j == T - 1),
                    )

                # phase 2, interleaved: agg_w = ps (cast bf16), rst = agg_w @ W
                agg = aggpool.tile([P, P], bf16, tag="agg")
                nc.scalar.copy(out=agg[:], in_=ps[:])
                ps2 = psum2pool.tile([P, D], f32)
                nc.tensor.matmul(
                    out=ps2[:], lhsT=agg[:], rhs=w_sb, start=True, stop=True
                )
                o = outpool.tile([P, D], f32)
                nc.vector.tensor_tensor(
                    out=o[:],
                    in0=ps2[:],
                    in1=s_sb[:, w : w + 1].to_broadcast([P, D]),
                    op=mult,
                )
                nc.vector.tensor_add(out=o[:], in0=o[:], in1=b_sb)
                nc.sync.dma_start(out=out_d[w * P : (w + 1) * P, :], in_=o[:])

    nc.compile()
    return nc


def kernel(h, src, dst, distance, weight, bias, _trace=False):
    import ml_dtypes
    from concourse.bass_utils import run_bass_kernel_spmd

    h = np.ascontiguousarray(np.asarray(h, dtype=np.float32))
    weight = np.asarray(weight, dtype=np.float32)
    bias = np.asarray(bias, dtype=np.float32)
    N, D = h.shape

    (
        table, idx16, rofs, coef, snode, out_core, out_row,
        n_windows, T_lo, T_hi, n_cols,
    ) = _prep_host(h, src, dst, distance, N_CORES)

    bf = ml_dtypes.bfloat16
    table16 = np.ascontiguousarray(table.astype(bf))
    iota = np.broadcast_to(np.arange(P, dtype=np.float32)[None, :], (P, P))
    biasf = np.broadcast_to(bias[None, :], (P, D)).astype(np.float32)

    nc = _build_nc(N, D, n_windows, T_lo, T_hi, n_cols)

    in_maps = []
    for c in range(N_CORES):
        fconst16 = np.concatenate(
            [rofs[c], coef[c], iota, weight], axis=1
        ).astype(bf)
        fconst32 = np.concatenate([biasf, snode[c]], axis=1).astype(np.float32)
        in_maps.append(
            {
                "h": table16,
                "idx16": np.ascontiguousarray(idx16[c]),
                "fconst16": np.ascontiguousarray(fconst16),
                "fconst32": np.ascontiguousarray(fconst32),
            }
        )

    res = run_bass_kernel_spmd(nc, in_maps, list(range(N_CORES)), trace=_trace)

    stacked = np.stack([res.results[c]["out"] for c in range(N_CORES)])
    out = stacked[out_core, out_row].astype(np.float32)

    if _trace:
        return out, res
    return out
